# revision 27
# baseline (speedup 1.0000x reference)
"""Sparse (tanh-clipped, key-masked) dot-product attention on 8 trn2 NeuronCores.

Reference computation (per batch b, head h):
    logits = (Q @ K^T) / 8
    logits = 10 * tanh(logits)
    logits[masked keys] = -inf          (mask is per (batch, key))
    out = softmax(logits) @ V

Strategy (v2 — ACT/DVE split of the softmax nonlinearity):
  - Host: gather K/V down to the unmasked keys (~50% of 2048), pad to a
    common multiple of 128 across batches.  Pre-transpose Q and K to
    [64, S] so the contraction dim (d=64) is on partitions.  The
    augmented-V "ones" column is the keep-indicator (0 on pad rows), so
    pad keys are inert in both the numerator and the denominator and no
    logit bias is needed at all.
  - Device (per core: one batch, 8 heads), per 128-key tile t:
      PE:  S_T[k, q] = Kt-tile^T @ Qt  (fp32r, into [128, 1024] PSUM
           half-tiles so PE/ACT pipeline without a 4-bank double buffer)
      ACT: t = tanh(s/8)  (one pass per half-tile, fp32 -> SBUF)
      then one of two prob paths (split to balance ACT vs DVE):
        A-tiles: ACT exp(10*t + beta)            -> P bf16
        B-tiles: DVE custom QUART_ANT (Horner deg-4 ~= e^{0.625 t}) then
                 DVE custom SQ16_ANT  (x^16)      -> P bf16
      The quartic satisfies quart(t)^16 = e^{10 t + g(t)} with
      |g - beta| <= 8e-4, and the constant beta cancels in softmax (the
      A-tile exp bias compensates it so both paths agree).
  - Phase 2 (per 128-query block, overlapped with the NEXT head's
    phase 1): PE out_q[128, 65] = sum_t P[t]^T @ [V|keep] (bf16), then
    GPSIMD (Pool) copies PSUM->SBUF and normalize_recip divides by the
    keep-column sum.  Output lands in natural [q, d] layout.
  - softmax needs no max-subtraction: 10*tanh is bounded in [-10, 10].
"""

import sys

if "/opt/trn_rl_repo" not in sys.path:
    sys.path.insert(0, "/opt/trn_rl_repo")

import ml_dtypes
import numpy as np

import concourse.tile as tile
from concourse import bacc, mybir
from concourse.bass_utils import run_bass_kernel_spmd

B, H, S, D = 4, 16, 2048, 64
N_CORES = 8
HPC = B * H // N_CORES  # heads per core = 8 (each core: 1 batch, 8 heads)
Q_CHUNK = 512  # fp32r moving-operand chunk (>=256 keeps 1 cycle/row)
HALF = 1024  # PSUM half-tile (2 banks) for PE/ACT pipelining
F32 = mybir.dt.float32
F32R = mybir.dt.float32r
BF16 = mybir.dt.bfloat16

# quart(t) = 1 + c1 t + c2 t^2 + c3 t^3 + c4 t^4 minimizes the spread of
# log(quart) - 0.625 t on [-1, 1] (9.8e-5); quart^16 ~= e^{10 t + BETA}.
QC1 = 0.6247876168305511
QC2 = 0.19549511897472363
QC3 = 0.04166201648849105
QC4 = 0.0062552006368568145
BETA = 0.00039008371296644917

# schedule-tuning knobs (set before _build_kernel; defaults = best sim result)
B_PROMOTE = False  # promote one extra tile to the DVE path on even heads
B_PROMOTE_IDX = 1
SPLIT_TILE = 3  # A-tile whose exp is split ACT/DVE by halves (-1: disabled)
TANH_BUFS = 3
QUART_BUFS = 2
ST_BUFS = 3

_kernel_cache = {}
_ops_cache = []


def _register_ops():
    """Register the two custom DVE ops (idempotent). Returns (QUART, SQ16)."""
    if _ops_cache:
        return _ops_cache
    from concourse import dve_ops
    from concourse.dve_spec import (
        C0,
        C1,
        C2,
        C3,
        One,
        Spec,
        Src0,
        _has_src1,
        _spill_c3_to_src1,
        lower,
        sq,
    )
    from concourse.dve_uop import DveOpSpec

    by_name = {o.name: o for o in dve_ops.OPS}
    if "QUART_ANT" in by_name:
        _ops_cache.extend([by_name["QUART_ANT"], by_name["SQ16_ANT"]])
        return _ops_cache

    t = Src0
    quart_body = _spill_c3_to_src1((((C0 * t + C1) * t + C2) * t + C3) * t + One)

    def quart_ref(in0, in1, c0, c1, c2):
        x = np.asarray(in0, np.float32)
        c3v = np.asarray(in1, np.float32).reshape(x.shape[0], 1)
        return ((((c0 * x + c1) * x + c2) * x + c3v) * x + 1.0).astype(np.float32)

    def sq16_ref(in0, in1, c0, c1, c2):
        x = np.asarray(in0, np.float32)
        for _ in range(4):
            x = x * x
        return x

    specs = [
        ("QUART_ANT", Spec(body=quart_body, reference=quart_ref)),
        ("SQ16_ANT", Spec(body=sq(sq(sq(sq(Src0)))), reference=sq16_ref)),
    ]
    for nm, spec in specs:
        row = dve_ops._CUSTOM_DVE_ROW_BASE + len(dve_ops.OPS)
        shas = {}
        for ver in ("v3", "v4"):
            uops = lower(spec, ver=ver)
            shas[ver] = DveOpSpec(
                name=nm, opcode=row, uops=uops, rd1_en=_has_src1(spec)
            ).sha(ver)
        op = dve_ops.DveOp(nm, spec, subdim=False, uops_sha=shas)
        dve_ops.OPS.append(op)
        dve_ops.CUSTOM_DVE_SPECS[nm] = spec
        dve_ops._SUB_OPCODE_FOR_NAME[nm] = row
        _ops_cache.append(op)
    return _ops_cache


def _build_kernel(n_kp: int, reps: int = 1):
    """Build the per-core Bass program for n_kp (padded) kept keys."""
    QUART, SQ16 = _register_ops()
    n_kt = n_kp // 128
    nc = bacc.Bacc(None)

    qt_p = nc.declare_dram_parameter("qt", [HPC, D, S], F32R, isOutput=False)
    kt_p = nc.declare_dram_parameter("kt", [HPC, D, n_kp], F32R, isOutput=False)
    v_p = nc.declare_dram_parameter("vaug", [HPC, 128, n_kt, D + 1], BF16, isOutput=False)
    out_p = nc.declare_dram_parameter("out", [HPC, S, D], F32, isOutput=True)

    n_qc = S // Q_CHUNK  # 4 q-chunks of 512
    n_qi = S // 128  # 16 query row-blocks
    n_hf = S // HALF  # 2 half-tiles per key tile
    # Per-tile prob path: "B" = DVE (quartic + sq16), "A" = ACT exp,
    # "S" = split (ACT exps half 0, DVE handles half 1). ACT cost:
    # ~1.0us/tanh-half + ~1.9us/exp; DVE: ~2.2us/custom inst. The exact
    # engine balance lands at ~3.7 ACT-exp-equivalents per head; alternate
    # two configs (3.5 and 4.0) to hit 3.75 on average without bursts.
    def modes_for(head_idx):
        m = ["B" if t % 2 == 0 else "A" for t in range(n_kt)]
        if n_kt >= 9:
            m[3] = "S"
            if head_idx % 2 == 1:
                m[8] = "S"
        return m

    with tile.TileContext(nc) as tc:
        with (
            tc.tile_pool(name="consts", bufs=1) as consts,
            tc.tile_pool(name="inq", bufs=2) as inq,
            tc.tile_pool(name="ink", bufs=2) as ink,
            tc.tile_pool(name="inv", bufs=2) as inv,
            tc.tile_pool(name="tanh", bufs=TANH_BUFS) as tanh_pool,
            tc.tile_pool(name="quart", bufs=QUART_BUFS) as quart_pool,
            # two headfuls of P tiles for full phase-1/phase-2 overlap;
            # capped for unusually low mask density (SBUF budget)
            tc.tile_pool(
                name="probs", bufs=2 * n_kt + 1 if n_kt <= 10 else n_kt + 4
            ) as probs_pool,
            tc.tile_pool(name="outsb", bufs=8) as out_pool,
            tc.tile_pool(name="ps_st", bufs=ST_BUFS, space="PSUM") as ps_st,
            tc.tile_pool(name="ps_oq", bufs=2, space="PSUM") as ps_oq,
        ):
            c1t = consts.tile([128, 1], F32)
            nc.vector.memset(c1t, QC1)
            beta_t = consts.tile([128, 1], F32)
            nc.vector.memset(beta_t, BETA)
            # prime the ACT exp_and_others table set (tanh+exp) while the
            # first input DMAs are still in flight
            warm = consts.tile([128, 1], F32)
            nc.scalar.activation(warm, c1t, mybir.ActivationFunctionType.Tanh)
            nc.scalar.activation(warm, warm, mybir.ActivationFunctionType.Exp)

            GP = 4  # q-blocks packed per PSUM accumulator tile (1 bank)

            def phase2_group(hh, g4, p_tiles, v_tile):
                # 4 independent accumulation chains packed into one PSUM bank,
                # evacuated by a single DVE copy (GPSIMD cannot read PSUM),
                # then normalized on the otherwise-idle Pool engine.
                oq_ps = ps_oq.tile([128, GP, D + 1], F32, tag="oq")
                for gi in range(GP):
                    qi = g4 * GP + gi
                    for j in range(n_kt):
                        nc.tensor.matmul(
                            oq_ps[:, gi, :],
                            lhsT=p_tiles[j][:, qi * 128 : (qi + 1) * 128],
                            rhs=v_tile[:, j, :],
                            start=(j == 0),
                            stop=(j == n_kt - 1),
                        )
                oq_sb = out_pool.tile([128, GP, D + 1], F32, tag="oqsb")
                nc.vector.tensor_copy(oq_sb, oq_ps)
                o_sb = out_pool.tile([128, GP, D], F32, tag="out")
                for gi in range(GP):
                    qi = g4 * GP + gi
                    nc.gpsimd.normalize_recip(
                        o_sb[:, gi, :],
                        oq_sb[:, gi, 0:D],
                        oq_sb[:, gi, D : D + 1],
                    )
                    # issue the output DMA from the Pool queue: it follows
                    # normalize_recip in-order with no cross-engine sem, and
                    # keeps the SP queue free to prefetch the next head's
                    # inputs instead of stalling behind phase-2 results
                    nc.gpsimd.dma_start(
                        out=out_p[hh, qi * 128 : (qi + 1) * 128, :],
                        in_=o_sb[:, gi, :],
                    )

            heads = [h for _ in range(reps) for h in range(HPC)]
            prev = None  # (head, p_tiles, v_tile) pending phase 2
            # spread the previous head's 4 packed phase-2 groups over this
            # head's key-tile steps (any leftovers drain after the loop)
            n_g4 = n_qi // GP
            per_step = -(-n_g4 // max(1, n_kt - 1))
            hc = [0]  # running half-tile counter for st_big slot rotation
            for i, h in enumerate(heads):
                qt_sb = inq.tile([D, S], F32R, tag="qt")
                kt_sb = ink.tile([D, n_kp], F32R, tag="kt")
                if i == 0:
                    # fine-grained first loads so the first matmul (and the
                    # ACT pipe behind it) starts as early as possible
                    nc.sync.dma_start(out=kt_sb[:, 0:128], in_=kt_p[h][:, 0:128])
                    for qc in range(n_qc):
                        nc.sync.dma_start(
                            out=qt_sb[:, qc * Q_CHUNK : (qc + 1) * Q_CHUNK],
                            in_=qt_p[h][:, qc * Q_CHUNK : (qc + 1) * Q_CHUNK],
                        )
                    if n_kp > 128:
                        nc.sync.dma_start(out=kt_sb[:, 128:], in_=kt_p[h][:, 128:])
                else:
                    nc.sync.dma_start(out=qt_sb, in_=qt_p[h])
                    nc.sync.dma_start(out=kt_sb, in_=kt_p[h])
                v_sb = inv.tile([128, n_kt, D + 1], BF16, tag="v")
                nc.sync.dma_start(out=v_sb, in_=v_p[h])

                p_tiles = []
                qi_cursor = 0
                modes = modes_for(i)
                for t in range(n_kt):
                    t_sb = tanh_pool.tile([128, S], F32, tag="tanh")
                    for hf in range(n_hf):
                        st_ps = ps_st.tile([128, HALF], F32, tag="st")
                        for qc in range(HALF // Q_CHUNK):
                            q0 = hf * HALF + qc * Q_CHUNK
                            nc.tensor.matmul(
                                st_ps[:, qc * Q_CHUNK : (qc + 1) * Q_CHUNK],
                                lhsT=kt_sb[:, t * 128 : (t + 1) * 128],
                                rhs=qt_sb[:, q0 : q0 + Q_CHUNK],
                                start=True,
                                stop=True,
                            )
                        nc.scalar.activation(
                            t_sb[:, hf * HALF : (hf + 1) * HALF],
                            st_ps,
                            mybir.ActivationFunctionType.Tanh,
                            scale=0.125,
                        )
                    p_sb = probs_pool.tile([128, S], BF16, tag="p")
                    if modes[t] == "B":
                        q_sb = quart_pool.tile([128, S], F32, tag="q")
                        nc.vector._custom_dve(
                            QUART, out=q_sb, in0=t_sb, in1=c1t,
                            s0=QC4, s1=QC3, imm2=QC2,
                        )
                        nc.vector._custom_dve(SQ16, out=p_sb, in0=q_sb)
                    elif modes[t] == "S":
                        # split tile: ACT exps half 0 while DVE handles half 1,
                        # evening out the ~5.5:3.5 optimum without bursts
                        nc.scalar.activation(
                            p_sb[:, 0:HALF],
                            t_sb[:, 0:HALF],
                            mybir.ActivationFunctionType.Exp,
                            bias=beta_t,
                            scale=10.0,
                        )
                        q_sb = quart_pool.tile([128, HALF], F32, tag="qh")
                        nc.vector._custom_dve(
                            QUART, out=q_sb, in0=t_sb[:, HALF:S], in1=c1t,
                            s0=QC4, s1=QC3, imm2=QC2,
                        )
                        nc.vector._custom_dve(SQ16, out=p_sb[:, HALF:S], in0=q_sb)
                    else:
                        nc.scalar.activation(
                            p_sb,
                            t_sb,
                            mybir.ActivationFunctionType.Exp,
                            bias=beta_t,
                            scale=10.0,
                        )
                    p_tiles.append(p_sb)
                    # overlap: drain the previous head's phase 2 under this
                    # head's phase-1 work
                    if prev is not None and t >= 1:
                        for _ in range(min(per_step, n_g4 - qi_cursor)):
                            phase2_group(prev[0], qi_cursor, prev[1], prev[2])
                            qi_cursor += 1
                if prev is not None:
                    for g4 in range(qi_cursor, n_g4):
                        phase2_group(prev[0], g4, prev[1], prev[2])
                prev = (h, p_tiles, v_sb)
            for g4 in range(n_g4):
                phase2_group(prev[0], g4, prev[1], prev[2])
    if not nc.is_finalized():
        nc.finalize()
    return nc


def _prep_inputs(q, k, v, mask):
    """Host-side shard + gather + layout. Returns (in_maps, n_kp)."""
    keep = [np.flatnonzero(~mask[b, :, 0]) for b in range(B)]
    n_kp = max(128, -(-max(len(kb) for kb in keep) // 128) * 128)
    n_kt = n_kp // 128

    in_maps = []
    for c in range(N_CORES):
        b = c // 2
        h0 = (c % 2) * HPC
        kb = keep[b]
        nk = len(kb)

        qt = np.ascontiguousarray(q[b, h0 : h0 + HPC].transpose(0, 2, 1))

        kg = np.zeros((HPC, n_kp, D), np.float32)
        kg[:, :nk] = k[b, h0 : h0 + HPC][:, kb]
        kt = np.ascontiguousarray(kg.transpose(0, 2, 1))

        vg = np.zeros((HPC, n_kp, D + 1), np.float32)
        vg[:, :nk, :D] = v[b, h0 : h0 + HPC][:, kb]
        vg[:, :nk, D] = 1.0  # keep-indicator: pad rows stay 0 => inert
        # [HPC, n_kt, 128, 65] -> [HPC, 128, n_kt, 65] (partition-major)
        vaug = np.ascontiguousarray(
            vg.reshape(HPC, n_kt, 128, D + 1).transpose(0, 2, 1, 3)
        ).astype(ml_dtypes.bfloat16)

        in_maps.append({"qt": qt, "kt": kt, "vaug": vaug})
    return in_maps, n_kp


def kernel(q, k, v, mask, _trace=False):
    q = np.asarray(q, np.float32)
    k = np.asarray(k, np.float32)
    v = np.asarray(v, np.float32)
    mask = np.asarray(mask, bool)
    assert q.shape == k.shape == v.shape == (B, H, S, D), (q.shape,)
    assert mask.shape == (B, S, 1), (mask.shape,)

    in_maps, n_kp = _prep_inputs(q, k, v, mask)
    if n_kp not in _kernel_cache:
        _kernel_cache[n_kp] = _build_kernel(n_kp)
    nc = _kernel_cache[n_kp]

    # a core occasionally comes up wedged (NRT_EXEC_UNIT_UNRECOVERABLE,
    # self-recovers in ~30 s) — retry rather than fail the whole call
    import time as _time

    res = None
    for attempt in range(3):
        try:
            res = run_bass_kernel_spmd(
                nc, in_maps, list(range(N_CORES)), trace=_trace
            )
            break
        except Exception:
            if attempt == 2:
                raise
            _time.sleep(30)
    out = np.empty((B, H, S, D), np.float32)
    for c in range(N_CORES):
        b = c // 2
        h0 = (c % 2) * HPC
        out[b, h0 : h0 + HPC] = res.results[c]["out"]
    if _trace:
        return out, res
    return out


if __name__ == "__main__":
    rng = np.random.default_rng(0)
    q = rng.standard_normal((B, H, S, D), np.float32)
    k = rng.standard_normal((B, H, S, D), np.float32)
    v = rng.standard_normal((B, H, S, D), np.float32)
    mask = rng.integers(0, 2, (B, S, 1)).astype(bool)
    out = kernel(q, k, v, mask)
    print("out", out.shape, out.dtype, float(np.abs(out).max()))
